# revision 8
# baseline (speedup 1.0000x reference)
"""Trainium2 Bass kernel for a Neural CDE forward pass.

Model (see reference): 2000 fixed Euler steps of
    y_{t+1} = y_t + dt * einsum('bhd,bd->bh', tanh-MLP(y_t).reshape(B,H,D), dX_t)
with a 3-layer softplus MLP (32 -> 128 -> 128 -> 256/tanh), batch B=128,
followed by a linear readout.

Strategy:
  * Pure data parallel over 8 NeuronCores (16 batch elements per core).
  * Feature-major activation layout (features on partitions, batch on the
    free dim) so every layer is a single PE matmul with a constant lhsT.
  * The cubic-spline derivative dX (times dt) is precomputed on the host
    for all 2000 steps, pre-broadcast to the 256-feature d-major layout,
    and streamed to SBUF in big chunks.
  * softplus(x) = Ln(Exp(x) + 1): two ScalarE ops from the single
    natural_log_exp activation table (gen3 has no softplus entry).
  * The whole tanh tail is ONE custom DVE op (TANH_RECIP_MUL_NCDE):
    ScalarE emits t3 = 0.5*exp(-2z) (the 0.5 riding the Exp bias), then
    the 8-stage DVE pipeline computes
        den = t3 + 0.5           (= (1+exp(-2z))/2)
        y   ~ 1/den              (exponent-flip seed + 1 Newton pass)
        g   = (y - 1)*c = tanh(z)*c      (c = dt*dX, streamed)
    Validated end-to-end in fp64/bit-exact sim: ~1.1e-3 rel err on logits.
  * y is never materialized per step.  PSUM bank `psum1` accumulates
    s_t = F0 @ y_t directly across all steps (psum1 += [A A A A] @ g_t),
    and the readout is logits = (R @ pinv(F0)) @ s_T + rb - no per-step
    Sel matmuls, no psum_y bank.
  * The activation-table registry is pinned so Exp/Ln/Identity resolve to
    the single natural_log_exp_and_others table (one ACT_TABLE_LOAD total).

Measured on trn2 (8 cores): ~2.47 us/step critical chain, 4.97 ms total
(vs 6.05 ms for the 3-DVE-op tail + psum_y baseline), rel err ~7e-4 vs
the fp32 reference.  Per-step chain: TRM -> MM1ab -> Exp/Ln -> MM2 ->
Exp/Ln -> MM3ab -> Exp -> TRM; ScalarE busy ~1.33 us of the 2.47 us
period, which is the floor given softplus needs the Exp+Ln table pair.
"""

import numpy as np

B = 128
NP_KNOTS = 128
D = 8
H = 32
WID = 128
NCLS = 10
T0, T1 = 0.0, 20.0
DT0 = 0.01
NUM_STEPS = 2000
NCORES = 8
BS = B // NCORES  # 16 batch per core

_F32 = np.float32

# Minimax constants for the 1-Newton exponent-flip reciprocal (see
# concourse.dve_ops.RECIP_APPROX_FAST_CONSTS derivation).
TRM_C0 = -0.235497943431996
TRM_C1 = 2.0017323506310354
TRM_NAME = "TANH_RECIP_MUL_NCDE"


# --------------------------------------------------------------------------
# Custom DVE op: g = (recip1(in0 + 0.5) * in1) - in1  ~= tanh(z) * in1
# --------------------------------------------------------------------------

def _register_trm_op():
    import concourse.dve_ops as dve_ops
    if TRM_NAME in dve_ops._SUB_OPCODE_FOR_NAME:
        return getattr(dve_ops, TRM_NAME)
    from concourse.dve_spec import AluOp, Bin, Spec, Src0, Src1, C0, C1, C2, lower
    from concourse.dve_spec import _has_src1
    from concourse.dve_uop import DveOpSpec
    from concourse.dve_table_gen import dve_ver_for

    den = Src0 + C2
    nx = Bin(AluOp.BITWISE_NOT, den, den)
    y0 = nx * C0
    y1 = y0 * (C1 - den * y0)
    body = y1 * Src1 - Src1

    def _ref(in0, in1, s0, s1, imm2):
        dn = (in0 + np.float32(imm2)).astype(np.float32)
        nxr = (~dn.view(np.int32)).view(np.float32)
        y0r = (nxr * np.float32(s0)).astype(np.float32)
        tr = (dn * y0r).astype(np.float32)
        ur = (np.float32(s1) - tr).astype(np.float32)
        y1r = (y0r * ur).astype(np.float32)
        return (y1r * in1).astype(np.float32) - in1

    spec = Spec(body=body, reference=_ref)
    ver = dve_ver_for("TRN2")
    row = max(dve_ops._SUB_OPCODE_FOR_NAME.values()) + 1
    assert row < 0x20
    sha = DveOpSpec(name=TRM_NAME, opcode=row, uops=lower(spec, ver=ver),
                    rd1_en=_has_src1(spec)).sha(ver)
    op = dve_ops.DveOp(TRM_NAME, spec, subdim=False, uops_sha={ver: sha})
    dve_ops.OPS.append(op)
    dve_ops.CUSTOM_DVE_SPECS[TRM_NAME] = spec
    dve_ops._SUB_OPCODE_FOR_NAME[TRM_NAME] = row
    setattr(dve_ops, TRM_NAME, op)
    return op


# --------------------------------------------------------------------------
# Host-side precompute
# --------------------------------------------------------------------------

def _spline_dx(ts, coeff_d, coeff_c, coeff_b, num_steps):
    """dX/dt at each Euler step start time, with the (clipped) dt folded in.

    Mirrors the reference computation in fp32.  Returns (S, B, D)."""
    t_grid = (ts[0] + _F32(DT0) * np.arange(num_steps, dtype=_F32)).astype(_F32)
    dts = np.minimum(_F32(DT0), ts[-1] - t_grid).astype(_F32)
    idx = np.clip(np.searchsorted(ts, t_grid, side="right") - 1, 0, NP_KNOTS - 2)
    fr = (t_grid - ts[idx]).astype(_F32)[None, :, None]
    dX = (coeff_b[:, idx] + _F32(2.0) * coeff_c[:, idx] * fr
          + _F32(3.0) * coeff_d[:, idx] * fr * fr)          # (B, S, D)
    dX = np.transpose(dX, (1, 0, 2)).astype(_F32)           # (S, B, D)
    return dX * dts[:, None, None]


def _dxb_layout(dx_core, steps_per_chunk):
    """(S, BS, D) -> (CH, 128, C*32) chunked, d-major, h-broadcast layout.

    Feature p in col-block cb holds global feature cb*128 + p,
    i.e. d = cb*4 + p//32, h = p % 32."""
    S = dx_core.shape[0]
    C = steps_per_chunk
    CH = S // C
    tmp = dx_core.reshape(S, BS, 2, 4)
    tmp = np.transpose(tmp, (0, 3, 2, 1))
    tmp = np.broadcast_to(tmp[:, :, None, :, :], (S, 4, 32, 2, BS))
    arr = tmp.reshape(S, 128, 32)                      # [s, p, cb*16 + j]
    arr = arr.reshape(CH, C, 128, 32).transpose(0, 2, 1, 3).reshape(CH, 128, C * 32)
    return np.ascontiguousarray(arr, dtype=_F32)


MM_DT = np.float16  # dtype of the per-step matmuls (fp16: 1 cyc/row + FWL)


def _host_weights(W0, b0, W1, b1, W2, b2, F0, f0, F1, f1, F2, f2, R, rb):
    """All constant tensors, already transposed/permuted for the kernel."""
    f32 = lambda a: np.ascontiguousarray(a, dtype=_F32)
    f16 = lambda a: np.ascontiguousarray(a, dtype=MM_DT)
    # d-major permutation of the 256 func-MLP output features
    p = np.arange(256)
    perm = (p % 32) * 8 + p // 32          # F2p[p] = F2[(p%32)*8 + p//32]
    F2p = F2[perm]
    f2p = f2[perm]
    # readout through the pseudo-inverse of F0: logits = (R pinv(F0)) s + rb
    MT = (R.astype(np.float64) @ np.linalg.pinv(F0.astype(np.float64))).T
    W = {
        "ATt":   f16(np.tile(F0.T, (4, 1))),          # (128,128) lhsT for psum1 += [A..A] @ g
        "F1T":   f16(F1.T),                            # (128,128)
        "F2aT":  f16(F2p[:128].T),                     # (128,128)
        "F2bT":  f16(F2p[128:].T),                     # (128,128)
        "f2rows": f16(np.stack([f2p[:128], f2p[128:]])),   # (2,128) bias lhsT
        "W0T":   f32(W0.T),                            # (8,128)
        "W1T":   f32(W1.T),                            # (128,128)
        "AW2T":  f32((F0 @ W2).T),                     # (128,128)
        "Ab2":   f32((F0 @ b2)[None, :]),              # (1,128)
        "MT":    f32(MT),                              # (128,10)
        "b0c":   f32(b0[:, None]),                     # (128,1)
        "b1c":   f32(b1[:, None]),
        "f0c":   f32(f0[:, None]),
        "f1c":   f32(f1[:, None]),
        "rbc":   f32(rb[:, None]),                     # (10,1)
        "lnhc":  f32(np.full((128, 1), -np.log(2.0))),  # (128,1) Exp bias

        "ones2": f16(np.stack([np.r_[np.ones(16), np.zeros(16)],
                               np.r_[np.zeros(16), np.ones(16)]])),  # (2,32)
        "ones16": f32(np.ones((1, 16))),
    }
    return W


# --------------------------------------------------------------------------
# Bass kernel build
# --------------------------------------------------------------------------

_NC_CACHE = {}


def _build_nc(num_steps, steps_per_chunk):
    key = (num_steps, steps_per_chunk)
    if key in _NC_CACHE:
        return _NC_CACHE[key]

    import concourse.bacc as bacc
    import concourse.bass as bass
    import concourse.mybir as mybir
    import concourse.tile as tile
    from contextlib import ExitStack

    trm_op = _register_trm_op()

    f32 = mybir.dt.float32
    mmdt = mybir.dt.from_np(np.dtype(MM_DT))
    AF = mybir.ActivationFunctionType

    # Pin the activation-function table (see baseline docstring): everything
    # we use (Exp, Ln, Identity) lives in natural_log_exp_and_others.
    import concourse.hw_specs as hw_specs
    _full_tabs = hw_specs.get_activation_tables("gen3")
    _ours = {AF.Exp, AF.Ln, AF.Identity, AF.Copy}
    _pinned = {
        name: (set(funcs) if name == "natural_log_exp_and_others"
               else set(funcs) - _ours)
        for name, funcs in _full_tabs.items()
    }
    bacc.get_activation_tables = lambda arch: _pinned

    S = num_steps
    C = steps_per_chunk
    assert S % C == 0
    CH = S // C

    nc = bacc.Bacc("TRN2", target_bir_lowering=False, debug=False)

    # ---- DRAM I/O ----
    dram = {}
    wshapes = {
        "ATt": (128, 128), "F1T": (128, 128), "F2aT": (128, 128),
        "F2bT": (128, 128), "f2rows": (2, 128),
        "W0T": (8, 128), "W1T": (128, 128),
        "AW2T": (128, 128), "Ab2": (1, 128), "MT": (128, 10),
        "b0c": (128, 1), "b1c": (128, 1), "f0c": (128, 1), "f1c": (128, 1),
        "rbc": (10, 1), "lnhc": (128, 1), "ones2": (2, 32), "ones16": (1, 16),
    }
    mm_names = {"ATt", "F1T", "F2aT", "F2bT", "f2rows", "ones2"}
    for name, shp in wshapes.items():
        dt_ = mmdt if name in mm_names else f32
        dram[name] = nc.dram_tensor(name, list(shp), dt_, kind="ExternalInput")
    dram["x0"] = nc.dram_tensor("x0", [8, BS], f32, kind="ExternalInput")
    dram["dxb"] = nc.dram_tensor("dxb", [CH, 128, C * 32], f32, kind="ExternalInput")
    out_dram = nc.dram_tensor("logits", [NCLS, BS], f32, kind="ExternalOutput")

    with tile.TileContext(nc) as tc, ExitStack() as ctx:
        const = ctx.enter_context(tc.tile_pool(name="const", bufs=1))
        dxbp = ctx.enter_context(tc.tile_pool(name="dxbp", bufs=2))
        work = ctx.enter_context(tc.tile_pool(name="work", bufs=4))
        psum = ctx.enter_context(
            tc.tile_pool(name="psum", bufs=1, space="PSUM"))
        ptmp = ctx.enter_context(
            tc.tile_pool(name="ptmp", bufs=2, space="PSUM"))

        # ---- constants into SBUF ----
        ct = {}
        for name, shp in wshapes.items():
            dt_ = mmdt if name in mm_names else f32
            ct[name] = const.tile(list(shp), dt_, tag=name, name=f"c_{name}")
            nc.sync.dma_start(ct[name][:], dram[name][:])
        x0_t = const.tile([8, BS], f32, tag="x0")
        nc.sync.dma_start(x0_t[:], dram["x0"][:])

        # ---- persistent PSUM tiles ----
        psum1 = psum.tile([128, BS], f32, tag="psum1")   # F0 @ y_t accumulator
        psum2 = psum.tile([128, BS], f32, tag="psum2")
        psum3 = psum.tile([128, 2 * BS], f32, tag="psum3")

        def softplus(ps_in, bias_ap, out_tile):
            """out = ln(1 + exp(ps_in + bias)); two ACT ops, one table."""
            e = ptmp.tile([128, BS], f32, tag="ptmp")
            nc.scalar.activation(e[:], ps_in, AF.Exp, bias=bias_ap)
            nc.scalar.activation(out_tile[:], e[:], AF.Ln, bias=1.0)

        # ---- initial MLP: y0 = W2 @ sp(W1 @ sp(W0 @ x0 + b0) + b1) + b2 ----
        psA = ptmp.tile([128, BS], f32, tag="ptmp")
        nc.tensor.matmul(psA[:], ct["W0T"][:], x0_t[:], start=True, stop=True)
        hA = work.tile([128, BS], f32, tag="h1")
        softplus(psA[:], ct["b0c"][:], hA)
        psB = ptmp.tile([128, BS], f32, tag="ptmp")
        nc.tensor.matmul(psB[:], ct["W1T"][:], hA[:], start=True, stop=True)
        hB = work.tile([128, BS], f32, tag="h2")
        softplus(psB[:], ct["b1c"][:], hB)

        # psum1 <- F0 @ y0 = (F0 @ W2) @ hB + F0 @ b2
        nc.tensor.matmul(psum1[:], ct["AW2T"][:], hB[:], start=True, stop=False,
                         skip_group_check=True)
        nc.tensor.matmul(psum1[:], ct["Ab2"][:], ct["ones16"][:],
                         start=False, stop=False, skip_group_check=True)

        # ---- the 2000-step Euler scan ----
        g_prev = None
        for ch in range(CH):
            dxb_t = dxbp.tile([128, C * 32], f32, tag="dxb")
            nc.sync.dma_start(dxb_t[:], dram["dxb"][ch])
            for c in range(C):
                t = ch * C + c
                if t > 0:
                    # psum1 += [A .. A] @ g_{t-1}   (both 128-col halves)
                    nc.tensor.matmul(psum1[:], ct["ATt"][:], g_prev[:, 0:BS],
                                     start=False, stop=False, skip_group_check=True)
                    nc.tensor.matmul(psum1[:], ct["ATt"][:], g_prev[:, BS:2 * BS],
                                     start=False, stop=False, skip_group_check=True)
                # layer 1: h1 = sp(psum1 + f0)
                h1 = work.tile([128, BS], mmdt, tag="h1s")
                softplus(psum1[:], ct["f0c"][:], h1)
                # layer 2
                nc.tensor.matmul(psum2[:], ct["F1T"][:], h1[:], start=True, stop=True)
                h2 = work.tile([128, BS], mmdt, tag="h2s")
                softplus(psum2[:], ct["f1c"][:], h2)
                # layer 3: psum3 = F2p @ h2 + f2p   (bias via K=2 matmul)
                nc.tensor.matmul(psum3[:], ct["f2rows"][:], ct["ones2"][:],
                                 start=True, stop=False, skip_group_check=True)
                nc.tensor.matmul(psum3[:, 0:BS], ct["F2aT"][:], h2[:],
                                 start=False, stop=False, skip_group_check=True)
                nc.tensor.matmul(psum3[:, BS:2 * BS], ct["F2bT"][:], h2[:],
                                 start=False, stop=True, skip_group_check=True)
                # tanh tail: t3 = 0.5*exp(-2z); g = tanh(z)*c in ONE DVE op
                t3 = work.tile([128, 2 * BS], f32, tag="t3")
                nc.scalar.activation(t3[:], psum3[:], AF.Exp, scale=-2.0,
                                     bias=ct["lnhc"][:])
                g = work.tile([128, 2 * BS], mmdt, tag="g")
                nc.vector._custom_dve(
                    trm_op, out=g[:], in0=t3[:],
                    in1=dxb_t[:, c * 32:(c + 1) * 32],
                    s0=TRM_C0, s1=TRM_C1, imm2=0.5)
                g_prev = g

        # ---- finish: s_T = F0 @ y_T ; logits = (R pinv(F0)) s_T + rb ----
        nc.tensor.matmul(psum1[:], ct["ATt"][:], g_prev[:, 0:BS],
                         start=False, stop=False, skip_group_check=True)
        nc.tensor.matmul(psum1[:], ct["ATt"][:], g_prev[:, BS:2 * BS],
                         start=False, stop=True, skip_group_check=True)
        s_sb = work.tile([128, BS], f32, tag="s_sb")
        nc.scalar.activation(s_sb[:], psum1[:], AF.Identity)
        psl = ptmp.tile([NCLS, BS], f32, tag="ptmp_l")
        nc.tensor.matmul(psl[:], ct["MT"][:], s_sb[:], start=True, stop=True)
        out_sb = work.tile([NCLS, BS], f32, tag="out_sb")
        nc.scalar.activation(out_sb[:], psl[:], AF.Identity, bias=ct["rbc"][:])
        nc.sync.dma_start(out_dram[:], out_sb[:])

    nc.compile()
    _NC_CACHE[key] = nc
    return nc


# --------------------------------------------------------------------------
# Public entry point
# --------------------------------------------------------------------------

def _prepare_inputs(ts, coeff_d, coeff_c, coeff_b, coeff_a,
                    W0, b0, W1, b1, W2, b2, F0, f0, F1, f1, F2, f2, R, rb,
                    num_steps, steps_per_chunk):
    ts = np.asarray(ts, dtype=_F32)
    coeff_a = np.asarray(coeff_a, dtype=_F32)
    dx = _spline_dx(ts, np.asarray(coeff_d, _F32), np.asarray(coeff_c, _F32),
                    np.asarray(coeff_b, _F32), num_steps)          # (S,B,D), dt folded
    W = _host_weights(*[np.asarray(a, _F32) for a in
                        (W0, b0, W1, b1, W2, b2, F0, f0, F1, f1, F2, f2, R, rb)])
    in_maps = []
    for core in range(NCORES):
        bs = slice(core * BS, (core + 1) * BS)
        m = dict(W)
        m["x0"] = np.ascontiguousarray(coeff_a[bs, 0, :].T)        # (8,16)
        m["dxb"] = _dxb_layout(dx[:, bs, :], steps_per_chunk)      # (CH,128,C*32)
        in_maps.append(m)
    return in_maps


def kernel(ts, coeff_d, coeff_c, coeff_b, coeff_a,
           W0, b0, W1, b1, W2, b2, F0, f0, F1, f1, F2, f2, R, rb):
    from concourse.bass_utils import run_bass_kernel_spmd

    num_steps = NUM_STEPS
    steps_per_chunk = 250
    nc = _build_nc(num_steps, steps_per_chunk)
    in_maps = _prepare_inputs(ts, coeff_d, coeff_c, coeff_b, coeff_a,
                              W0, b0, W1, b1, W2, b2, F0, f0, F1, f1, F2, f2,
                              R, rb, num_steps, steps_per_chunk)
    res = run_bass_kernel_spmd(nc, in_maps, list(range(NCORES)))
    logits = np.concatenate(
        [res.results[i]["logits"].T for i in range(NCORES)], axis=0)
    return np.ascontiguousarray(logits.astype(np.float32))


# revision 33
# speedup vs baseline: 1.0021x; 1.0021x over previous
"""Trainium2 Bass kernel for a Neural CDE forward pass.

Model (see reference): 2000 fixed Euler steps of
    y_{t+1} = y_t + dt * einsum('bhd,bd->bh', tanh-MLP(y_t).reshape(B,H,D), dX_t)
with a 3-layer softplus MLP (32 -> 128 -> 128 -> 256/tanh), batch B=128,
followed by a linear readout.

Strategy:
  * Pure data parallel over 8 NeuronCores (16 batch elements per core).
  * Feature-major activation layout (features on partitions, batch on the
    free dim) so every layer is a single PE matmul with a constant lhsT.
  * The cubic-spline derivative dX (times dt) is precomputed on the host
    for all 2000 steps, pre-broadcast to the 256-feature d-major layout,
    and streamed to SBUF in big chunks.
  * softplus(x) = Ln(Exp(x) + 1): two ScalarE ops from the single
    natural_log_exp activation table (gen3 has no softplus entry).
  * The whole tanh tail is ONE custom DVE op (TANH_RECIP_MUL_NCDE):
    ScalarE emits t3 = 0.5*exp(-2z) (the 0.5 riding the Exp bias), then
    the 8-stage DVE pipeline computes
        den = t3 + 0.5           (= (1+exp(-2z))/2)
        y   ~ 1/den              (exponent-flip seed + 1 Newton pass)
        g   = (y - 1)*c = tanh(z)*c      (c = dt*dX, streamed)
    Validated end-to-end in fp64/bit-exact sim: ~1.1e-3 rel err on logits.
  * y is never materialized per step.  PSUM bank `psum1` accumulates
    s_t = F0 @ y_t directly across all steps (psum1 += [A A A A] @ g_t),
    and the readout is logits = (R @ pinv(F0)) @ s_T + rb - no per-step
    Sel matmuls, no psum_y bank.
  * The activation-table registry is pinned so Exp/Ln/Identity resolve to
    the single natural_log_exp_and_others table (one ACT_TABLE_LOAD total).

Measured on trn2 (8 cores): ~2.47 us/step critical chain, 4.97 ms total
(vs 6.05 ms for the 3-DVE-op tail + psum_y baseline), rel err ~7e-4 vs
the fp32 reference.  Per-step chain: TRM -> MM1ab -> Exp/Ln -> MM2 ->
Exp/Ln -> MM3ab -> Exp -> TRM; ScalarE busy ~1.33 us of the 2.47 us
period, which is the floor given softplus needs the Exp+Ln table pair.
"""

import numpy as np

B = 128
NP_KNOTS = 128
D = 8
H = 32
WID = 128
NCLS = 10
T0, T1 = 0.0, 20.0
DT0 = 0.01
NUM_STEPS = 2000
NCORES = 8
BS = B // NCORES  # 16 batch per core

_F32 = np.float32

# Minimax constants for the 1-Newton exponent-flip reciprocal (see
# concourse.dve_ops.RECIP_APPROX_FAST_CONSTS derivation).
TRM_C0 = -0.235497943431996
TRM_C1 = 2.0017323506310354
TRM_NAME = "TANH_RECIP_MUL_NCDE"


# --------------------------------------------------------------------------
# Custom DVE op: g = (recip1(in0 + 0.5) * in1) - in1  ~= tanh(z) * in1
# --------------------------------------------------------------------------

def _register_trm_op():
    import concourse.dve_ops as dve_ops
    if TRM_NAME in dve_ops._SUB_OPCODE_FOR_NAME:
        return getattr(dve_ops, TRM_NAME)
    from concourse.dve_spec import AluOp, Bin, Spec, Src0, Src1, C0, C1, C2, lower
    from concourse.dve_spec import _has_src1
    from concourse.dve_uop import DveOpSpec
    from concourse.dve_table_gen import dve_ver_for

    den = Src0 + C2
    nx = Bin(AluOp.BITWISE_NOT, den, den)
    y0 = nx * C0
    y1 = y0 * (C1 - den * y0)
    body = y1 * Src1 - Src1

    def _ref(in0, in1, s0, s1, imm2):
        dn = (in0 + np.float32(imm2)).astype(np.float32)
        nxr = (~dn.view(np.int32)).view(np.float32)
        y0r = (nxr * np.float32(s0)).astype(np.float32)
        tr = (dn * y0r).astype(np.float32)
        ur = (np.float32(s1) - tr).astype(np.float32)
        y1r = (y0r * ur).astype(np.float32)
        return (y1r * in1).astype(np.float32) - in1

    spec = Spec(body=body, reference=_ref)
    ver = dve_ver_for("TRN2")
    row = max(dve_ops._SUB_OPCODE_FOR_NAME.values()) + 1
    assert row < 0x20
    sha = DveOpSpec(name=TRM_NAME, opcode=row, uops=lower(spec, ver=ver),
                    rd1_en=_has_src1(spec)).sha(ver)
    op = dve_ops.DveOp(TRM_NAME, spec, subdim=False, uops_sha={ver: sha})
    dve_ops.OPS.append(op)
    dve_ops.CUSTOM_DVE_SPECS[TRM_NAME] = spec
    dve_ops._SUB_OPCODE_FOR_NAME[TRM_NAME] = row
    setattr(dve_ops, TRM_NAME, op)
    return op


# --------------------------------------------------------------------------
# Host-side precompute
# --------------------------------------------------------------------------

def _spline_dx(ts, coeff_d, coeff_c, coeff_b, num_steps):
    """dX/dt at each Euler step start time, with the (clipped) dt folded in.

    Mirrors the reference computation in fp32.  Returns (S, B, D)."""
    t_grid = (ts[0] + _F32(DT0) * np.arange(num_steps, dtype=_F32)).astype(_F32)
    dts = np.minimum(_F32(DT0), ts[-1] - t_grid).astype(_F32)
    idx = np.clip(np.searchsorted(ts, t_grid, side="right") - 1, 0, NP_KNOTS - 2)
    fr = (t_grid - ts[idx]).astype(_F32)[None, :, None]
    dX = (coeff_b[:, idx] + _F32(2.0) * coeff_c[:, idx] * fr
          + _F32(3.0) * coeff_d[:, idx] * fr * fr)          # (B, S, D)
    dX = np.transpose(dX, (1, 0, 2)).astype(_F32)           # (S, B, D)
    return dX * dts[:, None, None]


DXB_FIRST = 32  # steps in the small first chunk (fast DMA ahead of step 0)


def _chunk_plan(num_steps, steps_per_chunk):
    """First chunk small so its DMA lands quickly and step 0 can start;
    the rest in equal big chunks (each a contiguous DRAM block)."""
    rest = num_steps - DXB_FIRST
    n = max(1, round(rest / steps_per_chunk))
    assert rest % n == 0, (num_steps, steps_per_chunk)
    plan = [(0, DXB_FIRST)]
    off = DXB_FIRST
    for _ in range(n):
        plan.append((off, rest // n))
        off += rest // n
    return plan


def _dxb_layout(dx_core, steps_per_chunk):
    """(S, BS, D) -> dxb0 (128, DXB_FIRST*32) + dxb (n, 128, C*32):
    d-major, h-broadcast layout, contiguous per chunk.

    Feature p in col-block cb holds global feature cb*128 + p,
    i.e. d = cb*4 + p//32, h = p % 32; column (in-chunk s)*32 + cb*16 + j."""
    S = dx_core.shape[0]
    tmp = dx_core.reshape(S, BS, 2, 4)
    tmp = np.transpose(tmp, (0, 3, 2, 1))
    tmp = np.broadcast_to(tmp[:, :, None, :, :], (S, 4, 32, 2, BS))
    arr = tmp.reshape(S, 128, 32)                      # [s, p, cb*16 + j]
    flat = np.transpose(arr, (1, 0, 2)).reshape(128, S * 32)
    plan = _chunk_plan(S, steps_per_chunk)
    dxb0 = np.ascontiguousarray(flat[:, :DXB_FIRST * 32], dtype=MM_DT)
    big = [flat[:, s * 32:(s + c) * 32] for s, c in plan[1:]]
    dxb = np.ascontiguousarray(np.stack(big), dtype=MM_DT)  # (n,128,C*32)
    return dxb0, dxb


MM_DT = np.float16  # dtype of the per-step matmuls (fp16: 1 cyc/row + FWL)


def _host_weights(W0, b0, W1, b1, W2, b2, F0, f0, F1, f1, F2, f2, R, rb):
    """All constant tensors, already transposed/permuted for the kernel."""
    f32 = lambda a: np.ascontiguousarray(a, dtype=_F32)
    f16 = lambda a: np.ascontiguousarray(a, dtype=MM_DT)
    # d-major permutation of the 256 func-MLP output features
    p = np.arange(256)
    perm = (p % 32) * 8 + p // 32          # F2p[p] = F2[(p%32)*8 + p//32]
    F2p = F2[perm]
    f2p = f2[perm]
    # readout through the pseudo-inverse of F0: logits = (R pinv(F0)) s + rb
    MT = (R.astype(np.float64) @ np.linalg.pinv(F0.astype(np.float64))).T
    # Pack the large 128-partition constants into two tensors so the
    # prologue issues 2 big DMAs instead of 12 small ones (each
    # DMA_DIRECT2D costs ~650ns of Sync-queue time ahead of step 0).
    # The small per-step lhsT tensors (f2rows, ones2) stay as compact
    # separate tiles: slicing them out of a wide pack makes their
    # per-step LDWEIGHTS strided and costs ~6ns/step.
    # packW16 cols: ATt | F1T | F2aT | F2bT                      (128,512)
    # packW32 cols: W1T | AW2T | MT | b0c b1c f0c f1c lnhc      (128,271)
    packW16 = np.concatenate(
        [np.tile(F0.T, (4, 1)), F1.T, F2p[:128].T, F2p[128:].T], axis=1)
    packW32 = np.concatenate(
        [W1.T, (F0 @ W2).T, MT,
         b0[:, None], b1[:, None], f0[:, None], f1[:, None],
         np.full((128, 1), -np.log(2.0))], axis=1)
    W = {
        "packW16": f16(packW16),                       # (128,512)
        "packW32": f32(packW32),                       # (128,271)
        "f2rows": f16(np.stack([f2p[:128], f2p[128:]])),   # (2,128) bias lhsT
        "W0T":   f32(W0.T),                            # (8,128)
        "Ab2":   f32((F0 @ b2)[None, :]),              # (1,128)
        "rbc":   f32(rb[:, None]),                     # (10,1)
        "ones2": f16(np.stack([np.r_[np.ones(16), np.zeros(16)],
                               np.r_[np.zeros(16), np.ones(16)]])),  # (2,32)
        "ones16": f32(np.ones((1, 16))),
    }
    return W


# --------------------------------------------------------------------------
# Bass kernel build
# --------------------------------------------------------------------------

_NC_CACHE = {}


def _build_nc(num_steps, steps_per_chunk):
    key = (num_steps, steps_per_chunk)
    if key in _NC_CACHE:
        return _NC_CACHE[key]

    import concourse.bacc as bacc
    import concourse.bass as bass
    import concourse.mybir as mybir
    import concourse.tile as tile
    from contextlib import ExitStack

    trm_op = _register_trm_op()

    f32 = mybir.dt.float32
    mmdt = mybir.dt.from_np(np.dtype(MM_DT))
    AF = mybir.ActivationFunctionType

    # Pin the activation-function table (see baseline docstring): everything
    # we use (Exp, Ln, Identity) lives in natural_log_exp_and_others.
    import concourse.hw_specs as hw_specs
    _full_tabs = hw_specs.get_activation_tables("gen3")
    _ours = {AF.Exp, AF.Ln, AF.Identity, AF.Copy}
    _pinned = {
        name: (set(funcs) if name == "natural_log_exp_and_others"
               else set(funcs) - _ours)
        for name, funcs in _full_tabs.items()
    }
    bacc.get_activation_tables = lambda arch: _pinned

    S = num_steps
    plan = _chunk_plan(num_steps, steps_per_chunk)

    nc = bacc.Bacc("TRN2", target_bir_lowering=False, debug=False)

    # ---- DRAM I/O ----
    dram = {}
    wshapes = {
        "packW16": (128, 512), "packW32": (128, 271), "f2rows": (2, 128),
        "W0T": (8, 128), "Ab2": (1, 128),
        "rbc": (10, 1), "ones2": (2, 32), "ones16": (1, 16),
    }
    mm_names = {"packW16", "f2rows", "ones2"}
    for name, shp in wshapes.items():
        dt_ = mmdt if name in mm_names else f32
        dram[name] = nc.dram_tensor(name, list(shp), dt_, kind="ExternalInput")
    dram["x0"] = nc.dram_tensor("x0", [8, BS], f32, kind="ExternalInput")
    dram["dxb0"] = nc.dram_tensor("dxb0", [128, DXB_FIRST * 32], mmdt,
                                  kind="ExternalInput")
    nbig, cbig = len(plan) - 1, plan[1][1]
    dram["dxb"] = nc.dram_tensor("dxb", [nbig, 128, cbig * 32], mmdt,
                                 kind="ExternalInput")
    out_dram = nc.dram_tensor("logits", [NCLS, BS], f32, kind="ExternalOutput")

    with tile.TileContext(nc) as tc, ExitStack() as ctx:
        const = ctx.enter_context(tc.tile_pool(name="const", bufs=1))
        dxbp = ctx.enter_context(tc.tile_pool(name="dxbp", bufs=2))
        work = ctx.enter_context(tc.tile_pool(name="work", bufs=4))
        psum = ctx.enter_context(
            tc.tile_pool(name="psum", bufs=1, space="PSUM"))
        ptmp = ctx.enter_context(
            tc.tile_pool(name="ptmp", bufs=2, space="PSUM"))

        # ---- constants into SBUF (order = Sync-queue order; step 0's
        # needs first, readout-only constants last) ----
        ct = {}
        for name in ("x0", "W0T", "packW32", "packW16", "f2rows", "ones2",
                     "ones16", "Ab2"):
            if name == "x0":
                x0_t = const.tile([8, BS], f32, tag="x0")
                nc.sync.dma_start(x0_t[:], dram["x0"][:])
                continue
            shp = wshapes[name]
            dt_ = mmdt if name in mm_names else f32
            ct[name] = const.tile(list(shp), dt_, tag=name, name=f"c_{name}")
            nc.sync.dma_start(ct[name][:], dram[name][:])
        # slice views into the packed constants
        pk16, pk32 = ct["packW16"], ct["packW32"]
        ct["ATt"] = pk16[:, 0:128]
        ct["F1T"] = pk16[:, 128:256]
        ct["F2aT"] = pk16[:, 256:384]
        ct["F2bT"] = pk16[:, 384:512]
        ct["W1T"] = pk32[:, 0:128]
        ct["AW2T"] = pk32[:, 128:256]
        ct["MT"] = pk32[:, 256:266]
        ct["b0c"] = pk32[:, 266:267]
        ct["b1c"] = pk32[:, 267:268]
        ct["f0c"] = pk32[:, 268:269]
        ct["f1c"] = pk32[:, 269:270]
        ct["lnhc"] = pk32[:, 270:271]

        # first (small) dxb chunk: issued right behind the critical weights
        # so step 0 isn't gated on a 4MB transfer
        dxb0_t = dxbp.tile([128, DXB_FIRST * 32], mmdt, tag="dxb0")
        nc.sync.dma_start(dxb0_t[:], dram["dxb0"][:])
        # readout-only constant can land late
        ct["rbc"] = const.tile([NCLS, 1], f32, tag="rbc", name="c_rbc")
        nc.sync.dma_start(ct["rbc"][:], dram["rbc"][:])

        # dummy activation: triggers the one-time ACT_TABLE_LOAD while the
        # weight DMAs are still in flight instead of on step 0's chain
        warm = work.tile([8, 1], f32, tag="warm")
        nc.scalar.activation(warm[:], x0_t[:, 0:1], AF.Exp)

        # ---- persistent PSUM tiles ----
        psum1 = psum.tile([128, BS], f32, tag="psum1")   # F0 @ y_t accumulator
        psum2 = psum.tile([128, BS], f32, tag="psum2")
        psum3 = psum.tile([128, 2 * BS], f32, tag="psum3")

        def softplus(ps_in, bias_ap, out_tile):
            """out = ln(1 + exp(ps_in + bias)); two ACT ops, one table."""
            e = ptmp.tile([128, BS], f32, tag="ptmp")
            nc.scalar.activation(e[:], ps_in, AF.Exp, bias=bias_ap)
            nc.scalar.activation(out_tile[:], e[:], AF.Ln, bias=1.0)

        # ---- initial MLP: y0 = W2 @ sp(W1 @ sp(W0 @ x0 + b0) + b1) + b2 ----
        psA = ptmp.tile([128, BS], f32, tag="ptmp")
        nc.tensor.matmul(psA[:], ct["W0T"][:], x0_t[:], start=True, stop=True)
        hA = work.tile([128, BS], f32, tag="h1")
        softplus(psA[:], ct["b0c"][:], hA)
        psB = ptmp.tile([128, BS], f32, tag="ptmp")
        nc.tensor.matmul(psB[:], ct["W1T"][:], hA[:], start=True, stop=True)
        hB = work.tile([128, BS], f32, tag="h2")
        softplus(psB[:], ct["b1c"][:], hB)

        # psum1 <- F0 @ y0 = (F0 @ W2) @ hB + F0 @ b2
        nc.tensor.matmul(psum1[:], ct["AW2T"][:], hB[:], start=True, stop=False,
                         skip_group_check=True)
        nc.tensor.matmul(psum1[:], ct["Ab2"][:], ct["ones16"][:],
                         start=False, stop=False, skip_group_check=True)

        # ---- the 2000-step Euler scan ----
        g_prev = None
        for ci, (start, cnt) in enumerate(plan):
            if ci == 0:
                dxb_t = dxb0_t
            else:
                dxb_t = dxbp.tile([128, cnt * 32], mmdt, tag="dxb")
                nc.sync.dma_start(dxb_t[:], dram["dxb"][ci - 1])
            for c in range(cnt):
                t = start + c
                if t > 0:
                    # psum1 += [A .. A] @ g_{t-1}   (both 128-col halves)
                    nc.tensor.matmul(psum1[:], ct["ATt"][:], g_prev[:, 0:BS],
                                     start=False, stop=False, skip_group_check=True)
                    nc.tensor.matmul(psum1[:], ct["ATt"][:], g_prev[:, BS:2 * BS],
                                     start=False, stop=False, skip_group_check=True)
                # layer 1: h1 = sp(psum1 + f0)
                h1 = work.tile([128, BS], mmdt, tag="h1s")
                softplus(psum1[:], ct["f0c"][:], h1)
                # layer 2
                nc.tensor.matmul(psum2[:], ct["F1T"][:], h1[:], start=True, stop=True)
                h2 = work.tile([128, BS], mmdt, tag="h2s")
                softplus(psum2[:], ct["f1c"][:], h2)
                # layer 3: psum3 = F2p @ h2 + f2p   (bias via K=2 matmul)
                nc.tensor.matmul(psum3[:], ct["f2rows"][:], ct["ones2"][:],
                                 start=True, stop=False, skip_group_check=True)
                nc.tensor.matmul(psum3[:, 0:BS], ct["F2aT"][:], h2[:],
                                 start=False, stop=False, skip_group_check=True)
                nc.tensor.matmul(psum3[:, BS:2 * BS], ct["F2bT"][:], h2[:],
                                 start=False, stop=True, skip_group_check=True)
                # tanh tail: t3 = 0.5*exp(-2z); g = tanh(z)*c in ONE DVE op
                t3 = work.tile([128, 2 * BS], f32, tag="t3")
                nc.scalar.activation(t3[:], psum3[:], AF.Exp, scale=-2.0,
                                     bias=ct["lnhc"][:])
                g = work.tile([128, 2 * BS], mmdt, tag="g")
                nc.vector._custom_dve(
                    trm_op, out=g[:], in0=t3[:],
                    in1=dxb_t[:, c * 32:(c + 1) * 32],
                    s0=TRM_C0, s1=TRM_C1, imm2=0.5)
                g_prev = g

        # ---- finish: s_T = F0 @ y_T ; logits = (R pinv(F0)) s_T + rb ----
        nc.tensor.matmul(psum1[:], ct["ATt"][:], g_prev[:, 0:BS],
                         start=False, stop=False, skip_group_check=True)
        nc.tensor.matmul(psum1[:], ct["ATt"][:], g_prev[:, BS:2 * BS],
                         start=False, stop=True, skip_group_check=True)
        s_sb = work.tile([128, BS], f32, tag="s_sb")
        nc.scalar.activation(s_sb[:], psum1[:], AF.Identity)
        psl = ptmp.tile([NCLS, BS], f32, tag="ptmp_l")
        nc.tensor.matmul(psl[:], ct["MT"][:], s_sb[:], start=True, stop=True)
        out_sb = work.tile([NCLS, BS], f32, tag="out_sb")
        nc.scalar.activation(out_sb[:], psl[:], AF.Identity, bias=ct["rbc"][:])
        nc.sync.dma_start(out_dram[:], out_sb[:])

    nc.compile()
    _NC_CACHE[key] = nc
    return nc


# --------------------------------------------------------------------------
# Public entry point
# --------------------------------------------------------------------------

def _prepare_inputs(ts, coeff_d, coeff_c, coeff_b, coeff_a,
                    W0, b0, W1, b1, W2, b2, F0, f0, F1, f1, F2, f2, R, rb,
                    num_steps, steps_per_chunk):
    ts = np.asarray(ts, dtype=_F32)
    coeff_a = np.asarray(coeff_a, dtype=_F32)
    dx = _spline_dx(ts, np.asarray(coeff_d, _F32), np.asarray(coeff_c, _F32),
                    np.asarray(coeff_b, _F32), num_steps)          # (S,B,D), dt folded
    W = _host_weights(*[np.asarray(a, _F32) for a in
                        (W0, b0, W1, b1, W2, b2, F0, f0, F1, f1, F2, f2, R, rb)])
    in_maps = []
    for core in range(NCORES):
        bs = slice(core * BS, (core + 1) * BS)
        m = dict(W)
        m["x0"] = np.ascontiguousarray(coeff_a[bs, 0, :].T)        # (8,16)
        m["dxb0"], m["dxb"] = _dxb_layout(dx[:, bs, :], steps_per_chunk)
        in_maps.append(m)
    return in_maps


def kernel(ts, coeff_d, coeff_c, coeff_b, coeff_a,
           W0, b0, W1, b1, W2, b2, F0, f0, F1, f1, F2, f2, R, rb):
    from concourse.bass_utils import run_bass_kernel_spmd

    num_steps = NUM_STEPS
    steps_per_chunk = 250
    nc = _build_nc(num_steps, steps_per_chunk)
    in_maps = _prepare_inputs(ts, coeff_d, coeff_c, coeff_b, coeff_a,
                              W0, b0, W1, b1, W2, b2, F0, f0, F1, f1, F2, f2,
                              R, rb, num_steps, steps_per_chunk)
    res = run_bass_kernel_spmd(nc, in_maps, list(range(NCORES)))
    logits = np.concatenate(
        [res.results[i]["logits"].T for i in range(NCORES)], axis=0)
    return np.ascontiguousarray(logits.astype(np.float32))


# revision 36
# speedup vs baseline: 1.0027x; 1.0006x over previous
"""Trainium2 Bass kernel for a Neural CDE forward pass.

Model (see reference): 2000 fixed Euler steps of
    y_{t+1} = y_t + dt * einsum('bhd,bd->bh', tanh-MLP(y_t).reshape(B,H,D), dX_t)
with a 3-layer softplus MLP (32 -> 128 -> 128 -> 256/tanh), batch B=128,
followed by a linear readout.

Strategy:
  * Pure data parallel over 8 NeuronCores (16 batch elements per core).
  * Feature-major activation layout (features on partitions, batch on the
    free dim) so every layer is a single PE matmul with a constant lhsT.
  * The cubic-spline derivative dX (times dt) is precomputed on the host
    for all 2000 steps, pre-broadcast to the 256-feature d-major layout,
    and streamed to SBUF in big chunks.
  * softplus(x) = Ln(Exp(x) + 1): two ScalarE ops from the single
    natural_log_exp activation table (gen3 has no softplus entry).
  * The whole tanh tail is ONE custom DVE op (TANH_RECIP_MUL_NCDE):
    ScalarE emits t3 = 0.5*exp(-2z) (the 0.5 riding the Exp bias), then
    the 8-stage DVE pipeline computes
        den = t3 + 0.5           (= (1+exp(-2z))/2)
        y   ~ 1/den              (exponent-flip seed + 1 Newton pass)
        g   = (y - 1)*c = tanh(z)*c      (c = dt*dX, streamed)
    Validated end-to-end in fp64/bit-exact sim: ~1.1e-3 rel err on logits.
  * y is never materialized per step.  PSUM bank `psum1` accumulates
    s_t = F0 @ y_t directly across all steps (psum1 += [A A A A] @ g_t),
    and the readout is logits = (R @ pinv(F0)) @ s_T + rb - no per-step
    Sel matmuls, no psum_y bank.
  * The activation-table registry is pinned so Exp/Ln/Identity resolve to
    the single natural_log_exp_and_others table (one ACT_TABLE_LOAD total).

Measured on trn2 (8 cores): ~2.47 us/step critical chain, ~4.96 ms total
(vs 6.05 ms for the 3-DVE-op tail + psum_y baseline), rel err ~7e-4 vs
the fp32 reference.  Per-step chain: TRM -> MM1ab -> Exp/Ln -> MM2 ->
Exp/Ln -> MM3ab -> Exp -> TRM; ScalarE busy ~1.33 us of the 2.47 us
period, which is the floor given softplus needs the Exp+Ln table pair.
Prologue trimmed 33us -> ~17us: weights packed into 2 DMAs, a small
(32-step) first dxb chunk ahead of the readout constants, fp16 dxb
stream, and a dummy Exp to hoist the one-time ACT_TABLE_LOAD off
step 0's chain.
"""

import numpy as np

B = 128
NP_KNOTS = 128
D = 8
H = 32
WID = 128
NCLS = 10
T0, T1 = 0.0, 20.0
DT0 = 0.01
NUM_STEPS = 2000
NCORES = 8
BS = B // NCORES  # 16 batch per core

_F32 = np.float32

# Minimax constants for the 1-Newton exponent-flip reciprocal (see
# concourse.dve_ops.RECIP_APPROX_FAST_CONSTS derivation).
TRM_C0 = -0.235497943431996
TRM_C1 = 2.0017323506310354
TRM_NAME = "TANH_RECIP_MUL_NCDE"


# --------------------------------------------------------------------------
# Custom DVE op: g = (recip1(in0 + 0.5) * in1) - in1  ~= tanh(z) * in1
# --------------------------------------------------------------------------

def _register_trm_op():
    import concourse.dve_ops as dve_ops
    if TRM_NAME in dve_ops._SUB_OPCODE_FOR_NAME:
        return getattr(dve_ops, TRM_NAME)
    from concourse.dve_spec import AluOp, Bin, Spec, Src0, Src1, C0, C1, C2, lower
    from concourse.dve_spec import _has_src1
    from concourse.dve_uop import DveOpSpec
    from concourse.dve_table_gen import dve_ver_for

    den = Src0 + C2
    nx = Bin(AluOp.BITWISE_NOT, den, den)
    y0 = nx * C0
    y1 = y0 * (C1 - den * y0)
    body = y1 * Src1 - Src1

    def _ref(in0, in1, s0, s1, imm2):
        dn = (in0 + np.float32(imm2)).astype(np.float32)
        nxr = (~dn.view(np.int32)).view(np.float32)
        y0r = (nxr * np.float32(s0)).astype(np.float32)
        tr = (dn * y0r).astype(np.float32)
        ur = (np.float32(s1) - tr).astype(np.float32)
        y1r = (y0r * ur).astype(np.float32)
        return (y1r * in1).astype(np.float32) - in1

    spec = Spec(body=body, reference=_ref)
    ver = dve_ver_for("TRN2")
    row = max(dve_ops._SUB_OPCODE_FOR_NAME.values()) + 1
    assert row < 0x20
    sha = DveOpSpec(name=TRM_NAME, opcode=row, uops=lower(spec, ver=ver),
                    rd1_en=_has_src1(spec)).sha(ver)
    op = dve_ops.DveOp(TRM_NAME, spec, subdim=False, uops_sha={ver: sha})
    dve_ops.OPS.append(op)
    dve_ops.CUSTOM_DVE_SPECS[TRM_NAME] = spec
    dve_ops._SUB_OPCODE_FOR_NAME[TRM_NAME] = row
    setattr(dve_ops, TRM_NAME, op)
    return op


# --------------------------------------------------------------------------
# Host-side precompute
# --------------------------------------------------------------------------

def _spline_dx(ts, coeff_d, coeff_c, coeff_b, num_steps):
    """dX/dt at each Euler step start time, with the (clipped) dt folded in.

    Mirrors the reference computation in fp32.  Returns (S, B, D)."""
    t_grid = (ts[0] + _F32(DT0) * np.arange(num_steps, dtype=_F32)).astype(_F32)
    dts = np.minimum(_F32(DT0), ts[-1] - t_grid).astype(_F32)
    idx = np.clip(np.searchsorted(ts, t_grid, side="right") - 1, 0, NP_KNOTS - 2)
    fr = (t_grid - ts[idx]).astype(_F32)[None, :, None]
    dX = (coeff_b[:, idx] + _F32(2.0) * coeff_c[:, idx] * fr
          + _F32(3.0) * coeff_d[:, idx] * fr * fr)          # (B, S, D)
    dX = np.transpose(dX, (1, 0, 2)).astype(_F32)           # (S, B, D)
    return dX * dts[:, None, None]


DXB_FIRST = 32  # steps in the small first chunk (fast DMA ahead of step 0)


def _chunk_plan(num_steps, steps_per_chunk):
    """First chunk small so its DMA lands quickly and step 0 can start;
    the rest in equal big chunks (each a contiguous DRAM block)."""
    rest = num_steps - DXB_FIRST
    n = max(1, round(rest / steps_per_chunk))
    assert rest % n == 0, (num_steps, steps_per_chunk)
    plan = [(0, DXB_FIRST)]
    off = DXB_FIRST
    for _ in range(n):
        plan.append((off, rest // n))
        off += rest // n
    return plan


def _dxb_layout(dx_core, steps_per_chunk):
    """(S, BS, D) -> dxb0 (128, DXB_FIRST*32) + dxb (n, 128, C*32):
    d-major, h-broadcast layout, contiguous per chunk.

    Feature p in col-block cb holds global feature cb*128 + p,
    i.e. d = cb*4 + p//32, h = p % 32; column (in-chunk s)*32 + cb*16 + j."""
    S = dx_core.shape[0]
    tmp = dx_core.reshape(S, BS, 2, 4)
    tmp = np.transpose(tmp, (0, 3, 2, 1))
    tmp = np.broadcast_to(tmp[:, :, None, :, :], (S, 4, 32, 2, BS))
    arr = tmp.reshape(S, 128, 32)                      # [s, p, cb*16 + j]
    flat = np.transpose(arr, (1, 0, 2)).reshape(128, S * 32)
    plan = _chunk_plan(S, steps_per_chunk)
    dxb0 = np.ascontiguousarray(flat[:, :DXB_FIRST * 32], dtype=MM_DT)
    big = [flat[:, s * 32:(s + c) * 32] for s, c in plan[1:]]
    dxb = np.ascontiguousarray(np.stack(big), dtype=MM_DT)  # (n,128,C*32)
    return dxb0, dxb


MM_DT = np.float16  # dtype of the per-step matmuls (fp16: 1 cyc/row + FWL)


def _host_weights(W0, b0, W1, b1, W2, b2, F0, f0, F1, f1, F2, f2, R, rb):
    """All constant tensors, already transposed/permuted for the kernel."""
    f32 = lambda a: np.ascontiguousarray(a, dtype=_F32)
    f16 = lambda a: np.ascontiguousarray(a, dtype=MM_DT)
    # d-major permutation of the 256 func-MLP output features
    p = np.arange(256)
    perm = (p % 32) * 8 + p // 32          # F2p[p] = F2[(p%32)*8 + p//32]
    F2p = F2[perm]
    f2p = f2[perm]
    # readout through the pseudo-inverse of F0: logits = (R pinv(F0)) s + rb
    MT = (R.astype(np.float64) @ np.linalg.pinv(F0.astype(np.float64))).T
    # Pack the large 128-partition constants into two tensors so the
    # prologue issues 2 big DMAs instead of 12 small ones (each
    # DMA_DIRECT2D costs ~650ns of Sync-queue time ahead of step 0).
    # The small per-step lhsT tensors (f2rows, ones2) stay as compact
    # separate tiles: slicing them out of a wide pack makes their
    # per-step LDWEIGHTS strided and costs ~6ns/step.
    # packW16 cols: ATt | F1T | F2aT | F2bT                      (128,512)
    # packW32 cols: W1T | AW2T | MT | b0c b1c f0c f1c lnhc      (128,271)
    packW16 = np.concatenate(
        [np.tile(F0.T, (4, 1)), F1.T, F2p[:128].T, F2p[128:].T], axis=1)
    packW32 = np.concatenate(
        [W1.T, (F0 @ W2).T, MT,
         b0[:, None], b1[:, None], f0[:, None], f1[:, None],
         np.full((128, 1), -np.log(2.0))], axis=1)
    W = {
        "packW16": f16(packW16),                       # (128,512)
        "packW32": f32(packW32),                       # (128,271)
        "f2rows": f16(np.stack([f2p[:128], f2p[128:]])),   # (2,128) bias lhsT
        "W0T":   f32(W0.T),                            # (8,128)
        "Ab2":   f32((F0 @ b2)[None, :]),              # (1,128)
        "rbc":   f32(rb[:, None]),                     # (10,1)
        "ones2": f16(np.stack([np.r_[np.ones(16), np.zeros(16)],
                               np.r_[np.zeros(16), np.ones(16)]])),  # (2,32)
        "ones16": f32(np.ones((1, 16))),
    }
    return W


# --------------------------------------------------------------------------
# Bass kernel build
# --------------------------------------------------------------------------

_NC_CACHE = {}


def _build_nc(num_steps, steps_per_chunk):
    key = (num_steps, steps_per_chunk)
    if key in _NC_CACHE:
        return _NC_CACHE[key]

    import concourse.bacc as bacc
    import concourse.bass as bass
    import concourse.mybir as mybir
    import concourse.tile as tile
    from contextlib import ExitStack

    trm_op = _register_trm_op()

    f32 = mybir.dt.float32
    mmdt = mybir.dt.from_np(np.dtype(MM_DT))
    AF = mybir.ActivationFunctionType

    # Pin the activation-function table (see baseline docstring): everything
    # we use (Exp, Ln, Identity) lives in natural_log_exp_and_others.
    import concourse.hw_specs as hw_specs
    _full_tabs = hw_specs.get_activation_tables("gen3")
    _ours = {AF.Exp, AF.Ln, AF.Identity, AF.Copy}
    _pinned = {
        name: (set(funcs) if name == "natural_log_exp_and_others"
               else set(funcs) - _ours)
        for name, funcs in _full_tabs.items()
    }
    bacc.get_activation_tables = lambda arch: _pinned

    S = num_steps
    plan = _chunk_plan(num_steps, steps_per_chunk)

    nc = bacc.Bacc("TRN2", target_bir_lowering=False, debug=False)

    # ---- DRAM I/O ----
    dram = {}
    wshapes = {
        "packW16": (128, 512), "packW32": (128, 271), "f2rows": (2, 128),
        "W0T": (8, 128), "Ab2": (1, 128),
        "rbc": (10, 1), "ones2": (2, 32), "ones16": (1, 16),
    }
    mm_names = {"packW16", "f2rows", "ones2"}
    for name, shp in wshapes.items():
        dt_ = mmdt if name in mm_names else f32
        dram[name] = nc.dram_tensor(name, list(shp), dt_, kind="ExternalInput")
    dram["x0"] = nc.dram_tensor("x0", [8, BS], f32, kind="ExternalInput")
    dram["dxb0"] = nc.dram_tensor("dxb0", [128, DXB_FIRST * 32], mmdt,
                                  kind="ExternalInput")
    nbig, cbig = len(plan) - 1, plan[1][1]
    dram["dxb"] = nc.dram_tensor("dxb", [nbig, 128, cbig * 32], mmdt,
                                 kind="ExternalInput")
    out_dram = nc.dram_tensor("logits", [NCLS, BS], f32, kind="ExternalOutput")

    with tile.TileContext(nc) as tc, ExitStack() as ctx:
        const = ctx.enter_context(tc.tile_pool(name="const", bufs=1))
        dxbp = ctx.enter_context(tc.tile_pool(name="dxbp", bufs=2))
        work = ctx.enter_context(tc.tile_pool(name="work", bufs=4))
        psum = ctx.enter_context(
            tc.tile_pool(name="psum", bufs=1, space="PSUM"))
        ptmp = ctx.enter_context(
            tc.tile_pool(name="ptmp", bufs=2, space="PSUM"))

        # ---- constants into SBUF (order = Sync-queue order; step 0's
        # needs first, readout-only constants last) ----
        ct = {}
        for name in ("x0", "W0T", "packW32", "packW16", "f2rows", "ones2",
                     "ones16", "Ab2"):
            if name == "x0":
                x0_t = const.tile([8, BS], f32, tag="x0")
                nc.sync.dma_start(x0_t[:], dram["x0"][:])
                continue
            shp = wshapes[name]
            dt_ = mmdt if name in mm_names else f32
            ct[name] = const.tile(list(shp), dt_, tag=name, name=f"c_{name}")
            nc.sync.dma_start(ct[name][:], dram[name][:])
        # slice views into the packed constants
        pk16, pk32 = ct["packW16"], ct["packW32"]
        ct["ATt"] = pk16[:, 0:128]
        ct["F1T"] = pk16[:, 128:256]
        ct["F2aT"] = pk16[:, 256:384]
        ct["F2bT"] = pk16[:, 384:512]
        ct["W1T"] = pk32[:, 0:128]
        ct["AW2T"] = pk32[:, 128:256]
        ct["MT"] = pk32[:, 256:266]
        ct["b0c"] = pk32[:, 266:267]
        ct["b1c"] = pk32[:, 267:268]
        ct["f0c"] = pk32[:, 268:269]
        ct["f1c"] = pk32[:, 269:270]
        ct["lnhc"] = pk32[:, 270:271]

        # first (small) dxb chunk: issued right behind the critical weights
        # so step 0 isn't gated on a 4MB transfer
        dxb0_t = dxbp.tile([128, DXB_FIRST * 32], mmdt, tag="dxb0")
        nc.sync.dma_start(dxb0_t[:], dram["dxb0"][:])
        # readout-only constant can land late
        ct["rbc"] = const.tile([NCLS, 1], f32, tag="rbc", name="c_rbc")
        nc.sync.dma_start(ct["rbc"][:], dram["rbc"][:])

        # dummy activation: triggers the one-time ACT_TABLE_LOAD while the
        # weight DMAs are still in flight instead of on step 0's chain
        warm = work.tile([8, 1], f32, tag="warm")
        nc.scalar.activation(warm[:], x0_t[:, 0:1], AF.Exp)

        # ---- persistent PSUM tiles ----
        psum1 = psum.tile([128, BS], f32, tag="psum1")   # F0 @ y_t accumulator
        psum2 = psum.tile([128, BS], f32, tag="psum2")
        psum3 = psum.tile([128, 2 * BS], f32, tag="psum3")

        def softplus(ps_in, bias_ap, out_tile):
            """out = ln(1 + exp(ps_in + bias)); two ACT ops, one table."""
            e = ptmp.tile([128, BS], f32, tag="ptmp")
            nc.scalar.activation(e[:], ps_in, AF.Exp, bias=bias_ap)
            nc.scalar.activation(out_tile[:], e[:], AF.Ln, bias=1.0)

        # ---- initial MLP: y0 = W2 @ sp(W1 @ sp(W0 @ x0 + b0) + b1) + b2 ----
        psA = ptmp.tile([128, BS], f32, tag="ptmp")
        nc.tensor.matmul(psA[:], ct["W0T"][:], x0_t[:], start=True, stop=True)
        hA = work.tile([128, BS], f32, tag="h1")
        softplus(psA[:], ct["b0c"][:], hA)
        psB = ptmp.tile([128, BS], f32, tag="ptmp")
        nc.tensor.matmul(psB[:], ct["W1T"][:], hA[:], start=True, stop=True)
        hB = work.tile([128, BS], f32, tag="h2")
        softplus(psB[:], ct["b1c"][:], hB)

        # psum1 <- F0 @ y0 = (F0 @ W2) @ hB + F0 @ b2
        nc.tensor.matmul(psum1[:], ct["AW2T"][:], hB[:], start=True, stop=False,
                         skip_group_check=True)
        nc.tensor.matmul(psum1[:], ct["Ab2"][:], ct["ones16"][:],
                         start=False, stop=False, skip_group_check=True)

        # ---- the 2000-step Euler scan ----
        g_prev = None
        for ci, (start, cnt) in enumerate(plan):
            if ci == 0:
                dxb_t = dxb0_t
            else:
                dxb_t = dxbp.tile([128, cnt * 32], mmdt, tag="dxb")
                nc.sync.dma_start(dxb_t[:], dram["dxb"][ci - 1])
            for c in range(cnt):
                t = start + c
                if t > 0:
                    # psum1 += [A .. A] @ g_{t-1}   (both 128-col halves)
                    nc.tensor.matmul(psum1[:], ct["ATt"][:], g_prev[:, 0:BS],
                                     start=False, stop=False, skip_group_check=True)
                    nc.tensor.matmul(psum1[:], ct["ATt"][:], g_prev[:, BS:2 * BS],
                                     start=False, stop=False, skip_group_check=True)
                # layer 1: h1 = sp(psum1 + f0)
                h1 = work.tile([128, BS], mmdt, tag="h1s")
                softplus(psum1[:], ct["f0c"][:], h1)
                # layer 2
                nc.tensor.matmul(psum2[:], ct["F1T"][:], h1[:], start=True, stop=True)
                h2 = work.tile([128, BS], mmdt, tag="h2s")
                softplus(psum2[:], ct["f1c"][:], h2)
                # layer 3: psum3 = F2p @ h2 + f2p   (bias via K=2 matmul)
                nc.tensor.matmul(psum3[:], ct["f2rows"][:], ct["ones2"][:],
                                 start=True, stop=False, skip_group_check=True)
                nc.tensor.matmul(psum3[:, 0:BS], ct["F2aT"][:], h2[:],
                                 start=False, stop=False, skip_group_check=True)
                nc.tensor.matmul(psum3[:, BS:2 * BS], ct["F2bT"][:], h2[:],
                                 start=False, stop=True, skip_group_check=True)
                # tanh tail: t3 = 0.5*exp(-2z); g = tanh(z)*c in ONE DVE op
                t3 = work.tile([128, 2 * BS], f32, tag="t3")
                nc.scalar.activation(t3[:], psum3[:], AF.Exp, scale=-2.0,
                                     bias=ct["lnhc"][:])
                g = work.tile([128, 2 * BS], mmdt, tag="g")
                nc.vector._custom_dve(
                    trm_op, out=g[:], in0=t3[:],
                    in1=dxb_t[:, c * 32:(c + 1) * 32],
                    s0=TRM_C0, s1=TRM_C1, imm2=0.5)
                g_prev = g

        # ---- finish: s_T = F0 @ y_T ; logits = (R pinv(F0)) s_T + rb ----
        nc.tensor.matmul(psum1[:], ct["ATt"][:], g_prev[:, 0:BS],
                         start=False, stop=False, skip_group_check=True)
        nc.tensor.matmul(psum1[:], ct["ATt"][:], g_prev[:, BS:2 * BS],
                         start=False, stop=True, skip_group_check=True)
        s_sb = work.tile([128, BS], f32, tag="s_sb")
        nc.scalar.activation(s_sb[:], psum1[:], AF.Identity)
        psl = ptmp.tile([NCLS, BS], f32, tag="ptmp_l")
        nc.tensor.matmul(psl[:], ct["MT"][:], s_sb[:], start=True, stop=True)
        out_sb = work.tile([NCLS, BS], f32, tag="out_sb")
        nc.scalar.activation(out_sb[:], psl[:], AF.Identity, bias=ct["rbc"][:])
        nc.sync.dma_start(out_dram[:], out_sb[:])

    nc.compile()
    _NC_CACHE[key] = nc
    return nc


# --------------------------------------------------------------------------
# Public entry point
# --------------------------------------------------------------------------

def _prepare_inputs(ts, coeff_d, coeff_c, coeff_b, coeff_a,
                    W0, b0, W1, b1, W2, b2, F0, f0, F1, f1, F2, f2, R, rb,
                    num_steps, steps_per_chunk):
    ts = np.asarray(ts, dtype=_F32)
    coeff_a = np.asarray(coeff_a, dtype=_F32)
    dx = _spline_dx(ts, np.asarray(coeff_d, _F32), np.asarray(coeff_c, _F32),
                    np.asarray(coeff_b, _F32), num_steps)          # (S,B,D), dt folded
    W = _host_weights(*[np.asarray(a, _F32) for a in
                        (W0, b0, W1, b1, W2, b2, F0, f0, F1, f1, F2, f2, R, rb)])
    in_maps = []
    for core in range(NCORES):
        bs = slice(core * BS, (core + 1) * BS)
        m = dict(W)
        m["x0"] = np.ascontiguousarray(coeff_a[bs, 0, :].T)        # (8,16)
        m["dxb0"], m["dxb"] = _dxb_layout(dx[:, bs, :], steps_per_chunk)
        in_maps.append(m)
    return in_maps


def kernel(ts, coeff_d, coeff_c, coeff_b, coeff_a,
           W0, b0, W1, b1, W2, b2, F0, f0, F1, f1, F2, f2, R, rb):
    from concourse.bass_utils import run_bass_kernel_spmd

    num_steps = NUM_STEPS
    steps_per_chunk = 250
    nc = _build_nc(num_steps, steps_per_chunk)
    in_maps = _prepare_inputs(ts, coeff_d, coeff_c, coeff_b, coeff_a,
                              W0, b0, W1, b1, W2, b2, F0, f0, F1, f1, F2, f2,
                              R, rb, num_steps, steps_per_chunk)
    res = run_bass_kernel_spmd(nc, in_maps, list(range(NCORES)))
    logits = np.concatenate(
        [res.results[i]["logits"].T for i in range(NCORES)], axis=0)
    return np.ascontiguousarray(logits.astype(np.float32))
